# revision 2
# baseline (speedup 1.0000x reference)
"""GAT layer (DGL GATConv + BatchNorm + ELU + residual) on 8 Trainium2 cores.

Slot-major design (v2, replaces the dma_gather/one-hot v1):
  - Shard dst nodes across 8 cores (12500/core). Per core, sort nodes by
    in-degree DESC and assign rank r -> (block r//128, slot r%128); within
    a block all 128 slots have near-equal degree, so padding each block to
    K = max-degree costs only ~8%.
  - Host pre-gathers x[src[e]] columns into xe [128ch, TOTCOL*128] f16:
    column (b, k, s) is the k-th in-edge source of slot s in block b,
    channel-major -- directly a matmul lhsT. Pad columns use a vector v
    with (W@almat)^T v = -300 (=> attention weight exp(-66) ~ 0) for slots
    with >= 1 edge, and zero columns (=> zero numerator => h = 0) for
    degree-0 slots.
  - Device, per (block, round of <=12 k): feat|el = xe_k^T @ Wf lands
    edge-major [slot, 136] in PSUM (4 banks x 3 windows); er rides in a
    spare PSUM window. w = exp(lrelu(el+er) - 6) = max(exp(x-6),
    exp(0.2x-6)) -- two scalar-engine exps + one DVE max (the hardware
    Lrelu act table does not honor alpha). DVE scales feat by w into msg
    c-major [slot, c*K + k] f16; GpSimd folds k in half twice, then one
    DVE tensor_reduce yields [slot, sum(w*feat) | sum w]. Reciprocal via
    scalar Ln/Exp batched once per 25 blocks (avoids act-table thrash).
  - No dma_gather (v1's 2.2ms GpSimd descriptor stream eliminated), no
    one-hot builds (v1's 2.5ms of DVE is_equal eliminated).
  - BatchNorm batch stats are computed on the host from h_out (f64);
    launch 2 applies the affine fold a*h+c, ELU and the residual in
    slot-major rows (no transposes).
"""
import sys
sys.path.insert(0, "/opt/trn_rl_repo")
import numpy as np

import concourse.bass as bass
import concourse.bacc as bacc
import concourse.mybir as mybir
import concourse.tile as tile
from concourse.bass_utils import run_bass_kernel_spmd

F32 = mybir.dt.float32
F16 = mybir.dt.float16

N = 100000
E = 1600000
IN_DIM = 128
H = 8
D = 16
HD = 128
ROW = IN_DIM + H          # 136: [feat | el]
NCORES = 8
NSHARD = 12500
NBLK = 100
SLOTS = NBLK * 128        # 12800
NEG_SLOPE = 0.2
EPS = 1e-5
WBIAS = -6.0              # exp bias; cancels in softmax, keeps f16 w finite
ELPAD = -300.0            # el value for pad columns (w ~ e^-66 after lrelu)

LAST_EXEC_NS = [0, 0]
_cache = {}


def _build_launch1(caps):
    """caps: [NBLK] edge-columns per block (multiples of 3, same all cores)."""
    caps = [int(k) for k in caps]
    coloff = np.zeros(NBLK + 1, np.int64)
    np.cumsum(caps, out=coloff[1:])
    KMAX = max(caps)

    nc = bacc.Bacc("TRN2", target_bir_lowering=False, debug=False,
                   num_devices=NCORES)
    totcol = int(coloff[-1])
    xe_d = nc.dram_tensor("xe", [128, totcol * 128], F16, kind="ExternalInput")
    xTp_d = nc.dram_tensor("xTp16", [128, SLOTS], F16, kind="ExternalInput")
    Wf_d = nc.dram_tensor("Wf", [128, ROW], F16, kind="ExternalInput")
    Wr_d = nc.dram_tensor("Wr", [128, H], F16, kind="ExternalInput")
    h_out = nc.dram_tensor("h_out", [SLOTS, HD], F32, kind="ExternalOutput")

    with tile.TileContext(nc) as tc:
        with (
            tc.tile_pool(name="const", bufs=1) as constp,
            tc.tile_pool(name="xe_sb", bufs=3) as xep,
            tc.tile_pool(name="xp_sb", bufs=3) as xpp,
            tc.tile_pool(name="msg", bufs=3) as msgp,
            tc.tile_pool(name="w1", bufs=4) as w1p,
            tc.tile_pool(name="fin", bufs=6) as finp,
            tc.tile_pool(name="fp_ps", bufs=2, space="PSUM") as fpp,
        ):
            Wf_sb = constp.tile([128, ROW], F16)
            nc.sync.dma_start(out=Wf_sb[:], in_=Wf_d[:])
            Wr_sb = constp.tile([128, H], F16)
            nc.sync.dma_start(out=Wr_sb[:], in_=Wr_d[:])
            wbias_col = constp.tile([128, 1], F32)
            nc.vector.memset(wbias_col[:], WBIAS)
            slope_col = constp.tile([128, 1], F32)
            nc.vector.memset(slope_col[:], NEG_SLOPE)
            eps_col = constp.tile([128, 1], F32)
            nc.vector.memset(eps_col[:], 1e-30)
            negone_col = constp.tile([128, 1], F32)
            nc.vector.memset(negone_col[:], -1.0)


            # per-block [num | sum_w] survives here until chunk finalize
            hs_all = constp.tile([128, NBLK * ROW], F16)
            CHUNK = 25

            for b in range(NBLK):
                K = caps[b]
                nrounds = (K + 11) // 12
                # ---- streams in ----
                xe_sb = xep.tile([128, KMAX * 128], F16, tag="xe")
                q = nc.sync if b % 2 == 0 else nc.gpsimd
                q.dma_start(out=xe_sb[:, :K * 128],
                            in_=xe_d[:, coloff[b] * 128:(coloff[b] + K) * 128])
                xp_sb = xpp.tile([128, 128], F16, tag="xp")
                nc.gpsimd.dma_start(out=xp_sb[:],
                                    in_=xTp_d[:, b * 128:(b + 1) * 128])
                msg = msgp.tile([128, ROW * KMAX], F16, tag="msg")
                # c-major views of the active region [128, ROW*K]
                msg_ck = msg[:, :ROW * K].rearrange("p (c k) -> p c k", k=K)

                for r in range(nrounds):
                    k0 = 12 * r
                    nk = min(12, K - k0)         # 3, 6, 9 or 12
                    nb = nk // 3                 # PSUM banks used (1..4)
                    fp = fpp.tile([128, 2048], F32, tag="fp")
                    for j in range(nk):
                        off = (j // 3) * 512 + (j % 3) * ROW
                        nc.tensor.matmul(
                            out=fp[:, off:off + ROW],
                            lhsT=xe_sb[:, (k0 + j) * 128:(k0 + j + 1) * 128],
                            rhs=Wf_sb[:], start=True, stop=True)
                    # er for this block's slots -> spare tail of bank 0
                    nc.tensor.matmul(out=fp[:, 3 * ROW:3 * ROW + H],
                                     lhsT=xp_sb[:], rhs=Wr_sb[:],
                                     start=True, stop=True)
                    er_sb = w1p.tile([128, H], F32, tag="er_sb")
                    nc.scalar.activation(er_sb[:], fp[:, 3 * ROW:3 * ROW + H],
                                         mybir.ActivationFunctionType.Copy)
                    # [p, bank, j(3), c(136)] view of the PSUM round
                    fp4 = (fp[:].rearrange("p (bk x) -> p bk x", bk=4)
                           [:, 0:nb, 0:3 * ROW]
                           .rearrange("p bk (j c) -> p bk j c", c=ROW))
                    # el + er -> w1, (h, k) layout within a 9-wide tile
                    w1 = w1p.tile([128, 12 * H], F32, tag="w1")
                    w1_kh = w1[:].rearrange("p (h k) -> p k h", k=12)
                    nc.vector.tensor_tensor(
                        out=w1_kh[:].rearrange("p (bk j) h -> p bk j h",
                                               j=3)[:, 0:nb],
                        in0=fp4[:, :, :, IN_DIM:ROW],
                        in1=er_sb[:].rearrange("p (o t h) -> p o t h",
                                               o=1, t=1)
                            .to_broadcast([128, nb, 3, H]),
                        op=mybir.AluOpType.add)
                    # w = exp(lrelu(x) - 6) = max(exp(x-6), exp(0.2x-6))
                    # (exp is monotone), both exps on the scalar engine
                    w1_hk = w1[:].rearrange("p (h k) -> p h k", k=12)
                    nc.scalar.activation(
                        msg_ck[:, IN_DIM:ROW, k0:k0 + nk],
                        w1_hk[:, :, 0:nk],
                        mybir.ActivationFunctionType.Exp,
                        bias=wbias_col[:])
                    w2 = w1p.tile([128, 12 * H], F16, tag="w2")
                    w2_hk = w2[:].rearrange("p (h k) -> p h k", k=12)
                    nc.scalar.activation(
                        w2_hk[:, :, 0:nk],
                        w1_hk[:, :, 0:nk],
                        mybir.ActivationFunctionType.Exp,
                        bias=wbias_col[:], scale=slope_col[:])
                    nc.vector.tensor_tensor(
                        out=msg_ck[:, IN_DIM:ROW, k0:k0 + nk],
                        in0=msg_ck[:, IN_DIM:ROW, k0:k0 + nk],
                        in1=w2_hk[:, :, 0:nk],
                        op=mybir.AluOpType.max)
                    # msg[c, k] = feat * w  (w broadcast over d=16)
                    msg_hdk = msg[:, :IN_DIM * K].rearrange(
                        "p (h d k) -> p h d k", d=D, k=K)
                    w_bc = (msg_ck[:, IN_DIM:ROW, k0:k0 + nk]
                            .rearrange("p h (o bk j) -> p h o bk j",
                                       o=1, j=3)
                            .to_broadcast([128, H, D, nb, 3]))
                    out_hdk = msg_hdk[:, :, :, k0:k0 + nk].rearrange(
                        "p h d (bk j) -> p h d bk j", j=3)
                    nc.vector.tensor_tensor(
                        out=out_hdk,
                        in0=fp4[:, :, :, 0:IN_DIM].rearrange(
                            "p bk j (h d) -> p h d bk j", d=D),
                        in1=w_bc,
                        op=mybir.AluOpType.mult)

                # ---- reduce over k: [slot, 136] = [sum w*feat | sum w] ----
                # gpsimd folds the top half onto the bottom half, DVE
                # reduces the remainder
                m = K // 2
                rem = K - m
                if m > 0:
                    with nc.allow_low_precision(reason="f16 softmax accum"):
                        nc.gpsimd.tensor_tensor(
                            out=msg_ck[:, :, 0:m],
                            in0=msg_ck[:, :, 0:m],
                            in1=msg_ck[:, :, K - m:K],
                            op=mybir.AluOpType.add)
                m2 = rem // 2
                if m2 > 0:
                    with nc.allow_low_precision(reason="f16 softmax accum"):
                        nc.gpsimd.tensor_tensor(
                            out=msg_ck[:, :, 0:m2],
                            in0=msg_ck[:, :, 0:m2],
                            in1=msg_ck[:, :, rem - m2:rem],
                            op=mybir.AluOpType.add)
                rem = rem - m2
                with nc.allow_low_precision(reason="f16 softmax accum"):
                    nc.vector.tensor_reduce(
                        out=hs_all[:, b * ROW:(b + 1) * ROW],
                        in_=msg_ck[:, :, 0:rem],
                        axis=mybir.AxisListType.X,
                        op=mybir.AluOpType.add)

                if b % CHUNK != CHUNK - 1:
                    continue
                # ---- chunk finalize for blocks [b0, b] ----
                b0 = b - (CHUNK - 1)
                hs_c = hs_all[:, b0 * ROW:(b + 1) * ROW].rearrange(
                    "p (t c) -> p t c", c=ROW)
                lnw = finp.tile([128, CHUNK * H], F32, tag="lnw")
                nc.scalar.activation(
                    lnw[:].rearrange("p (t h) -> p t h", h=H),
                    hs_c[:, :, IN_DIM:ROW],
                    mybir.ActivationFunctionType.Ln,
                    bias=eps_col[:])
                rec = finp.tile([128, CHUNK * H], F32, tag="rec")
                nc.scalar.activation(rec[:], lnw[:],
                                     mybir.ActivationFunctionType.Exp,
                                     scale=negone_col[:])
                for t in range(CHUNK):
                    bb = b0 + t
                    h_sb = finp.tile([128, HD], F32, tag="h")
                    nc.vector.tensor_tensor(
                        out=h_sb[:].rearrange("p (h d) -> p h d", d=D),
                        in0=hs_all[:, bb * ROW:bb * ROW + IN_DIM].rearrange(
                            "p (h d) -> p h d", d=D),
                        in1=rec[:, t * H:(t + 1) * H].rearrange(
                            "p (h o) -> p h o", o=1)
                            .to_broadcast([128, H, D]),
                        op=mybir.AluOpType.mult)
                    nc.sync.dma_start(out=h_out[bb * 128:(bb + 1) * 128, :],
                                      in_=h_sb[:])


    nc.compile()
    return nc


def _build_launch2():
    """Slot-major: BN affine + ELU + residual, no transposes."""
    nc = bacc.Bacc("TRN2", target_bir_lowering=False, debug=False,
                   num_devices=NCORES)
    h_in = nc.dram_tensor("h_in", [SLOTS, HD], F32, kind="ExternalInput")
    x_rows = nc.dram_tensor("x_rows", [SLOTS, HD], F32, kind="ExternalInput")
    acr = nc.dram_tensor("acr", [1, 2 * HD], F32, kind="ExternalInput")
    out_r = nc.dram_tensor("out_r", [SLOTS, HD], F32, kind="ExternalOutput")

    CH = 1024         # slots per chunk (8 x 128-row tiles)
    NT = CH // 128
    chunks = [(i * CH) for i in range(SLOTS // CH)]
    if SLOTS % CH:
        chunks.append(SLOTS - (SLOTS % CH))

    with tile.TileContext(nc) as tc:
        with (
            tc.tile_pool(name="const", bufs=1) as constp,
            tc.tile_pool(name="ld", bufs=3) as ldp,
            tc.tile_pool(name="ps", bufs=1, space="PSUM") as psp,
            tc.tile_pool(name="wk", bufs=3) as wkp,
        ):
            ones_row = constp.tile([1, 128], F32)
            nc.vector.memset(ones_row[:], 1.0)
            acr_sb = constp.tile([1, 2 * HD], F32)
            nc.sync.dma_start(out=acr_sb[:], in_=acr[:])
            ac_ps = psp.tile([128, 2 * HD], F32)
            nc.tensor.matmul(out=ac_ps[:], lhsT=ones_row[:], rhs=acr_sb[:],
                             start=True, stop=True)
            abc = constp.tile([128, 2 * HD], F32)
            nc.scalar.activation(abc[:], ac_ps[:],
                                 mybir.ActivationFunctionType.Copy)
            a_b = (abc[:, 0:HD].rearrange("p (o c) -> p o c", o=1)
                   .to_broadcast([128, NT, HD]))
            c_b = (abc[:, HD:2 * HD].rearrange("p (o c) -> p o c", o=1)
                   .to_broadcast([128, NT, HD]))

            qs = [nc.sync, nc.gpsimd, nc.scalar]
            for i, o in enumerate(chunks):
                w = min(CH, SLOTS - o)
                nt = w // 128
                hl = ldp.tile([128, CH], F32, tag="hl")
                qs[i % 3].dma_start(
                    out=hl[:, :w].rearrange("p (f c) -> p f c", c=HD),
                    in_=h_in[o:o + w, :].rearrange("(f p) c -> p f c", f=nt))
                h2 = wkp.tile([128, CH], F16, tag="h2")
                h2v = h2[:, :w].rearrange("p (f c) -> p f c", c=HD)
                nc.vector.tensor_tensor(
                    out=h2v, in0=hl[:, :w].rearrange("p (f c) -> p f c",
                                                     c=HD),
                    in1=a_b[:, 0:nt], op=mybir.AluOpType.mult)
                nc.vector.tensor_tensor(
                    out=h2v, in0=h2v, in1=c_b[:, 0:nt],
                    op=mybir.AluOpType.add)
                m = wkp.tile([128, CH], F16, tag="m")
                nc.vector.tensor_scalar(out=m[:, :w], in0=h2[:, :w],
                                        scalar1=0.0, scalar2=None,
                                        op0=mybir.AluOpType.min)
                nc.scalar.activation(m[:, :w], m[:, :w],
                                     mybir.ActivationFunctionType.Exp)
                nc.vector.tensor_scalar(out=m[:, :w], in0=m[:, :w],
                                        scalar1=-1.0, scalar2=None,
                                        op0=mybir.AluOpType.add)
                # elu = max(h2, exp(min(h2,0))-1)
                nc.vector.tensor_tensor(out=h2[:, :w], in0=h2[:, :w],
                                        in1=m[:, :w],
                                        op=mybir.AluOpType.max)
                xt = ldp.tile([128, CH], F32, tag="xt")
                qs[(i + 1) % 3].dma_start(
                    out=xt[:, :w].rearrange("p (f c) -> p f c", c=HD),
                    in_=x_rows[o:o + w, :].rearrange("(f p) c -> p f c",
                                                     f=nt))
                ho = wkp.tile([128, CH], F32, tag="ho")
                nc.vector.tensor_tensor(out=ho[:, :w], in0=h2[:, :w],
                                        in1=xt[:, :w],
                                        op=mybir.AluOpType.add)
                qs[(i + 2) % 3].dma_start(
                    out=out_r[o:o + w, :].rearrange("(f p) c -> p f c",
                                                    f=nt),
                    in_=ho[:, :w].rearrange("p (f c) -> p f c", c=HD))

    nc.compile()
    return nc


def _host_prep(src, dst):
    """Degree-sorted slot assignment + per-edge column placement.

    Returns (per_core, caps, coloff):
      per_core[c] = (colsrc [totcol, 128] int32 index into xext columns,
                     node_of_slot [SLOTS] int64, -1 for pads)
      caps [NBLK], coloff [NBLK+1]
    """
    src = src.astype(np.int64)
    dst = dst.astype(np.int64)
    tmp = []
    caps_all = np.zeros((NCORES, NBLK), np.int64)
    for c in range(NCORES):
        lo = c * NSHARD
        m = (dst >= lo) & (dst < lo + NSHARD)
        es = src[m]
        ed = dst[m] - lo
        deg = np.bincount(ed, minlength=NSHARD)
        order = np.argsort(-deg, kind="stable")
        rank = np.empty(NSHARD, np.int64)
        rank[order] = np.arange(NSHARD)
        dpad = np.zeros(SLOTS, np.int64)
        dpad[:NSHARD] = deg[order]
        caps_all[c] = dpad.reshape(NBLK, 128).max(axis=1)
        r_e = rank[ed]
        eorder = np.argsort(r_e, kind="stable")
        r_sorted = r_e[eorder]
        s_sorted = es[eorder]
        starts = np.searchsorted(r_sorted, np.arange(NSHARD))
        k_e = np.arange(len(r_sorted)) - starts[r_sorted]
        tmp.append((order, dpad, r_sorted, s_sorted, k_e))
    caps = caps_all.max(axis=0)
    caps = np.maximum(((caps + 2) // 3) * 3, 3)
    coloff = np.zeros(NBLK + 1, np.int64)
    np.cumsum(caps, out=coloff[1:])
    totcol = int(coloff[-1])

    per_core = []
    for c in range(NCORES):
        order, dpad, r_sorted, s_sorted, k_e = tmp[c]
        node_of_slot = np.full(SLOTS, -1, np.int64)
        node_of_slot[:NSHARD] = order + c * NSHARD
        # defaults: v-pad (col N) where slot has edges, zero (col N+1) else
        defaults = np.where(dpad.reshape(NBLK, 128) > 0, N, N + 1)
        colsrc = np.repeat(defaults, caps, axis=0).astype(np.int32)
        b_e = r_sorted // 128
        s_e = r_sorted % 128
        colpos = coloff[b_e] + k_e
        colsrc[colpos, s_e] = s_sorted.astype(np.int32)
        per_core.append((colsrc, node_of_slot))
    return per_core, caps, coloff


def kernel(x, src, dst, W, attn_l, attn_r, bias, gamma, beta):
    global LAST_EXEC_NS
    x = np.asarray(x, np.float32)
    src = np.asarray(src, np.int32)
    dst = np.asarray(dst, np.int32)
    W = np.asarray(W, np.float32)
    attn_l = np.asarray(attn_l, np.float32)
    attn_r = np.asarray(attn_r, np.float32)
    gamma = np.asarray(gamma, np.float32)
    beta = np.asarray(beta, np.float32)

    per_core, caps, coloff = _host_prep(src, dst)

    key = ("l1", caps.tobytes())
    if key not in _cache:
        _cache[key] = _build_launch1(caps)
    if "l2" not in _cache:
        _cache["l2"] = _build_launch2()
    nc1, nc2 = _cache[key], _cache["l2"]

    almat = np.zeros((HD, H), np.float32)
    armat = np.zeros((HD, H), np.float32)
    for h in range(H):
        almat[h * D:(h + 1) * D, h] = attn_l[h]
        armat[h * D:(h + 1) * D, h] = attn_r[h]
    Wl = W @ almat                                       # [128, 8]
    Wf = np.concatenate([W, Wl], axis=1).astype(np.float16)   # [128, 136]
    Wr = (W @ armat).astype(np.float16)                  # [128, 8]
    # pad vector: Wl^T v = ELPAD * ones  (least squares, exact: 8 <= 128)
    G = Wl.T @ Wl
    v = Wl @ np.linalg.solve(G, np.full(H, ELPAD, np.float64)).astype(
        np.float32)
    xext = np.concatenate(
        [x.T.astype(np.float16),
         v[:, None].astype(np.float16),
         np.zeros((HD, 1), np.float16)], axis=1)         # [128, N+2]

    in_maps = []
    xrows_list = []
    for c in range(NCORES):
        colsrc, node_of_slot = per_core[c]
        xe = xext[:, colsrc.reshape(-1)]                 # [128, totcol*128]
        xr = np.zeros((SLOTS, HD), np.float32)
        real = node_of_slot >= 0
        xr[real] = x[node_of_slot[real]]
        xrows_list.append(xr)
        in_maps.append({
            "xe": xe, "xTp16": xr.T.astype(np.float16),
            "Wf": Wf, "Wr": Wr,
        })

    res1 = run_bass_kernel_spmd(nc1, in_maps, list(range(NCORES)),
                                **_trace_kwargs())
    LAST_EXEC_NS[0] = res1.exec_time_ns or 0

    # host: combine BN stats from h (pad slots are exactly zero)
    S1 = np.zeros(128, np.float64)
    S2 = np.zeros(128, np.float64)
    for c in range(NCORES):
        hv = res1.results[c]["h_out"].astype(np.float64)
        S1 += hv.sum(axis=0)
        S2 += (hv * hv).sum(axis=0)
    mu = (S1 / N).astype(np.float32)
    var = (S2 / N - (S1 / N) ** 2).astype(np.float32)
    a = gamma / np.sqrt(var + EPS)
    cc = beta - a * mu
    ac = np.stack([a, cc], axis=1).astype(np.float32)

    acr = np.concatenate([a, cc]).reshape(1, 2 * HD).astype(np.float32)
    in_maps2 = []
    for c in range(NCORES):
        in_maps2.append({
            "h_in": res1.results[c]["h_out"],
            "x_rows": xrows_list[c],
            "acr": acr,
        })
    res2 = run_bass_kernel_spmd(nc2, in_maps2, list(range(NCORES)),
                                **_trace_kwargs())
    LAST_EXEC_NS[1] = res2.exec_time_ns or 0

    out = np.zeros((N, IN_DIM), np.float32)
    for c in range(NCORES):
        node_of_slot = per_core[c][1]
        real = node_of_slot >= 0
        orow = res2.results[c]["out_r"]
        out[node_of_slot[real]] = orow[real]
    return out


def _trace_kwargs():
    import os
    if os.environ.get("GAT_TRACE", "0") == "1":
        return {"trace": True}
    return {}


# revision 3
# speedup vs baseline: 1.0341x; 1.0341x over previous
"""GAT layer (DGL GATConv + BatchNorm + ELU + residual) on 8 Trainium2 cores.

Slot-major design (v2, replaces the dma_gather/one-hot v1):
  - Shard dst nodes across 8 cores (12500/core). Per core, sort nodes by
    in-degree DESC and assign rank r -> (block r//128, slot r%128); within
    a block all 128 slots have near-equal degree, so padding each block to
    K = max-degree costs only ~8%.
  - Host pre-gathers x[src[e]] columns into xe [128ch, TOTCOL*128] f16:
    column (b, k, s) is the k-th in-edge source of slot s in block b,
    channel-major -- directly a matmul lhsT. Pad columns use a vector v
    with (W@almat)^T v = -300 (=> attention weight exp(-66) ~ 0) for slots
    with >= 1 edge, and zero columns (=> zero numerator => h = 0) for
    degree-0 slots.
  - Device, per (block, round of <=12 k): feat|el = xe_k^T @ Wf lands
    edge-major [slot, 136] in PSUM (4 banks x 3 windows); er rides in a
    spare PSUM window. w = exp(lrelu(el+er) - 6) = max(exp(x-6),
    exp(0.2x-6)) -- two scalar-engine exps + one DVE max (the hardware
    Lrelu act table does not honor alpha). DVE scales feat by w into msg
    c-major [slot, c*K + k] f16; GpSimd folds k in half twice, then one
    DVE tensor_reduce yields [slot, sum(w*feat) | sum w]. Reciprocal via
    scalar Ln/Exp batched once per 25 blocks (avoids act-table thrash).
  - No dma_gather (v1's 2.2ms GpSimd descriptor stream eliminated), no
    one-hot builds (v1's 2.5ms of DVE is_equal eliminated).
  - BatchNorm batch stats are computed on the host from h_out (f64);
    launch 2 applies the affine fold a*h+c, ELU and the residual in
    slot-major rows (no transposes).
"""
import sys
sys.path.insert(0, "/opt/trn_rl_repo")
import numpy as np

import concourse.bass as bass
import concourse.bacc as bacc
import concourse.mybir as mybir
import concourse.tile as tile
from concourse.bass_utils import run_bass_kernel_spmd

F32 = mybir.dt.float32
F16 = mybir.dt.float16

N = 100000
E = 1600000
IN_DIM = 128
H = 8
D = 16
HD = 128
ROW = IN_DIM + H          # 136: [feat | el]
NCORES = 8
NSHARD = 12500
NBLK = 100
SLOTS = NBLK * 128        # 12800
NEG_SLOPE = 0.2
EPS = 1e-5
WBIAS = -6.0              # exp bias; cancels in softmax, keeps f16 w finite
ELPAD = -300.0            # el value for pad columns (w ~ e^-66 after lrelu)

LAST_EXEC_NS = [0, 0]
_cache = {}


def _build_launch1(caps):
    """caps: [NBLK] edge-columns per block (multiples of 3, same all cores)."""
    caps = [int(k) for k in caps]
    coloff = np.zeros(NBLK + 1, np.int64)
    np.cumsum(caps, out=coloff[1:])
    KMAX = max(caps)

    nc = bacc.Bacc("TRN2", target_bir_lowering=False, debug=False,
                   num_devices=NCORES)
    totcol = int(coloff[-1])
    xe_d = nc.dram_tensor("xe", [128, totcol * 128], F16, kind="ExternalInput")
    xTp_d = nc.dram_tensor("xTp16", [128, SLOTS], F16, kind="ExternalInput")
    Wf_d = nc.dram_tensor("Wf", [128, ROW], F16, kind="ExternalInput")
    Wr_d = nc.dram_tensor("Wr", [128, H], F16, kind="ExternalInput")
    h_out = nc.dram_tensor("h_out", [SLOTS, HD], F32, kind="ExternalOutput")

    with tile.TileContext(nc) as tc:
        with (
            tc.tile_pool(name="const", bufs=1) as constp,
            tc.tile_pool(name="xe_sb", bufs=3) as xep,
            tc.tile_pool(name="xp_sb", bufs=3) as xpp,
            tc.tile_pool(name="msg", bufs=3) as msgp,
            tc.tile_pool(name="w1", bufs=4) as w1p,
            tc.tile_pool(name="fin", bufs=6) as finp,
            tc.tile_pool(name="fp_ps", bufs=2, space="PSUM") as fpp,
        ):
            Wf_sb = constp.tile([128, ROW], F16)
            nc.sync.dma_start(out=Wf_sb[:], in_=Wf_d[:])
            Wr_sb = constp.tile([128, H], F16)
            nc.sync.dma_start(out=Wr_sb[:], in_=Wr_d[:])
            wbias_col = constp.tile([128, 1], F32)
            nc.vector.memset(wbias_col[:], WBIAS)
            slope_col = constp.tile([128, 1], F32)
            nc.vector.memset(slope_col[:], NEG_SLOPE)
            eps_col = constp.tile([128, 1], F32)
            nc.vector.memset(eps_col[:], 1e-30)
            negone_col = constp.tile([128, 1], F32)
            nc.vector.memset(negone_col[:], -1.0)


            # per-block [num | sum_w] survives here until chunk finalize
            hs_all = constp.tile([128, NBLK * ROW], F16)
            CHUNK = 25

            for b in range(NBLK):
                K = caps[b]
                nrounds = (K + 11) // 12
                # ---- streams in ----
                xe_sb = xep.tile([128, KMAX * 128], F16, tag="xe")
                q = nc.sync if b % 2 == 0 else nc.gpsimd
                q.dma_start(out=xe_sb[:, :K * 128],
                            in_=xe_d[:, coloff[b] * 128:(coloff[b] + K) * 128])
                xp_sb = xpp.tile([128, 128], F16, tag="xp")
                nc.gpsimd.dma_start(out=xp_sb[:],
                                    in_=xTp_d[:, b * 128:(b + 1) * 128])
                msg = msgp.tile([128, ROW * KMAX], F16, tag="msg")
                # c-major views of the active region [128, ROW*K]
                msg_ck = msg[:, :ROW * K].rearrange("p (c k) -> p c k", k=K)

                def emit_front(r):
                    """matmuls + el-add + exps for round r; returns state."""
                    k0 = 12 * r
                    nk = min(12, K - k0)         # 3, 6, 9 or 12
                    nb = nk // 3                 # PSUM banks used (1..4)
                    fp = fpp.tile([128, 2048], F32, tag="fp")
                    for j in range(nk):
                        off = (j // 3) * 512 + (j % 3) * ROW
                        nc.tensor.matmul(
                            out=fp[:, off:off + ROW],
                            lhsT=xe_sb[:, (k0 + j) * 128:(k0 + j + 1) * 128],
                            rhs=Wf_sb[:], start=True, stop=True)
                    # er for this block's slots -> spare tail of bank 0
                    nc.tensor.matmul(out=fp[:, 3 * ROW:3 * ROW + H],
                                     lhsT=xp_sb[:], rhs=Wr_sb[:],
                                     start=True, stop=True)
                    er_sb = w1p.tile([128, H], F32, tag="er_sb")
                    nc.scalar.activation(er_sb[:], fp[:, 3 * ROW:3 * ROW + H],
                                         mybir.ActivationFunctionType.Copy)
                    # [p, bank, j(3), c(136)] view of the PSUM round
                    fp4 = (fp[:].rearrange("p (bk x) -> p bk x", bk=4)
                           [:, 0:nb, 0:3 * ROW]
                           .rearrange("p bk (j c) -> p bk j c", c=ROW))
                    # el + er -> w1, (h, k) layout
                    w1 = w1p.tile([128, 12 * H], F32, tag="w1")
                    w1_kh = w1[:].rearrange("p (h k) -> p k h", k=12)
                    nc.vector.tensor_tensor(
                        out=w1_kh[:].rearrange("p (bk j) h -> p bk j h",
                                               j=3)[:, 0:nb],
                        in0=fp4[:, :, :, IN_DIM:ROW],
                        in1=er_sb[:].rearrange("p (o t h) -> p o t h",
                                               o=1, t=1)
                            .to_broadcast([128, nb, 3, H]),
                        op=mybir.AluOpType.add)
                    # w = exp(lrelu(x) - 6) = max(exp(x-6), exp(0.2x-6))
                    # (exp is monotone), both exps on the scalar engine
                    w1_hk = w1[:].rearrange("p (h k) -> p h k", k=12)
                    nc.scalar.activation(
                        msg_ck[:, IN_DIM:ROW, k0:k0 + nk],
                        w1_hk[:, :, 0:nk],
                        mybir.ActivationFunctionType.Exp,
                        bias=wbias_col[:])
                    w2 = w1p.tile([128, 12 * H], F16, tag="w2")
                    w2_hk = w2[:].rearrange("p (h k) -> p h k", k=12)
                    nc.scalar.activation(
                        w2_hk[:, :, 0:nk],
                        w1_hk[:, :, 0:nk],
                        mybir.ActivationFunctionType.Exp,
                        bias=wbias_col[:], scale=slope_col[:])
                    return (k0, nk, nb, fp4, w2_hk)

                def emit_back(st):
                    """max + feat*w scale for a prepared round."""
                    k0, nk, nb, fp4, w2_hk = st
                    nc.vector.tensor_tensor(
                        out=msg_ck[:, IN_DIM:ROW, k0:k0 + nk],
                        in0=msg_ck[:, IN_DIM:ROW, k0:k0 + nk],
                        in1=w2_hk[:, :, 0:nk],
                        op=mybir.AluOpType.max)
                    msg_hdk = msg[:, :IN_DIM * K].rearrange(
                        "p (h d k) -> p h d k", d=D, k=K)
                    w_bc = (msg_ck[:, IN_DIM:ROW, k0:k0 + nk]
                            .rearrange("p h (o bk j) -> p h o bk j",
                                       o=1, j=3)
                            .to_broadcast([128, H, D, nb, 3]))
                    out_hdk = msg_hdk[:, :, :, k0:k0 + nk].rearrange(
                        "p h d (bk j) -> p h d bk j", j=3)
                    nc.vector.tensor_tensor(
                        out=out_hdk,
                        in0=fp4[:, :, :, 0:IN_DIM].rearrange(
                            "p bk j (h d) -> p h d bk j", d=D),
                        in1=w_bc,
                        op=mybir.AluOpType.mult)

                # process rounds in pairs so the DVE queue holds the next
                # round's el-add while this round's exps run on the scalar
                # engine (fp bufs=2 caps the pair depth)
                r = 0
                while r < nrounds:
                    st0 = emit_front(r)
                    st1 = emit_front(r + 1) if r + 1 < nrounds else None
                    emit_back(st0)
                    if st1 is not None:
                        emit_back(st1)
                    r += 2

                # ---- reduce over k: [slot, 136] = [sum w*feat | sum w] ----
                # gpsimd folds the top half onto the bottom half, DVE
                # reduces the remainder
                m = K // 2
                rem = K - m
                if m > 0:
                    with nc.allow_low_precision(reason="f16 softmax accum"):
                        nc.gpsimd.tensor_tensor(
                            out=msg_ck[:, :, 0:m],
                            in0=msg_ck[:, :, 0:m],
                            in1=msg_ck[:, :, K - m:K],
                            op=mybir.AluOpType.add)
                m2 = rem // 2
                if m2 > 0:
                    with nc.allow_low_precision(reason="f16 softmax accum"):
                        nc.gpsimd.tensor_tensor(
                            out=msg_ck[:, :, 0:m2],
                            in0=msg_ck[:, :, 0:m2],
                            in1=msg_ck[:, :, rem - m2:rem],
                            op=mybir.AluOpType.add)
                rem = rem - m2
                with nc.allow_low_precision(reason="f16 softmax accum"):
                    nc.vector.tensor_reduce(
                        out=hs_all[:, b * ROW:(b + 1) * ROW],
                        in_=msg_ck[:, :, 0:rem],
                        axis=mybir.AxisListType.X,
                        op=mybir.AluOpType.add)

                if b % CHUNK != CHUNK - 1:
                    continue
                # ---- chunk finalize for blocks [b0, b] ----
                b0 = b - (CHUNK - 1)
                hs_c = hs_all[:, b0 * ROW:(b + 1) * ROW].rearrange(
                    "p (t c) -> p t c", c=ROW)
                lnw = finp.tile([128, CHUNK * H], F32, tag="lnw")
                nc.scalar.activation(
                    lnw[:].rearrange("p (t h) -> p t h", h=H),
                    hs_c[:, :, IN_DIM:ROW],
                    mybir.ActivationFunctionType.Ln,
                    bias=eps_col[:])
                rec = finp.tile([128, CHUNK * H], F32, tag="rec")
                nc.scalar.activation(rec[:], lnw[:],
                                     mybir.ActivationFunctionType.Exp,
                                     scale=negone_col[:])
                for t in range(CHUNK):
                    bb = b0 + t
                    h_sb = finp.tile([128, HD], F32, tag="h")
                    nc.vector.tensor_tensor(
                        out=h_sb[:].rearrange("p (h d) -> p h d", d=D),
                        in0=hs_all[:, bb * ROW:bb * ROW + IN_DIM].rearrange(
                            "p (h d) -> p h d", d=D),
                        in1=rec[:, t * H:(t + 1) * H].rearrange(
                            "p (h o) -> p h o", o=1)
                            .to_broadcast([128, H, D]),
                        op=mybir.AluOpType.mult)
                    nc.sync.dma_start(out=h_out[bb * 128:(bb + 1) * 128, :],
                                      in_=h_sb[:])


    nc.compile()
    return nc


def _build_launch2():
    """Slot-major: BN affine + ELU + residual, no transposes."""
    nc = bacc.Bacc("TRN2", target_bir_lowering=False, debug=False,
                   num_devices=NCORES)
    h_in = nc.dram_tensor("h_in", [SLOTS, HD], F32, kind="ExternalInput")
    x_rows = nc.dram_tensor("x_rows", [SLOTS, HD], F32, kind="ExternalInput")
    acr = nc.dram_tensor("acr", [1, 2 * HD], F32, kind="ExternalInput")
    out_r = nc.dram_tensor("out_r", [SLOTS, HD], F32, kind="ExternalOutput")

    CH = 1024         # slots per chunk (8 x 128-row tiles)
    NT = CH // 128
    chunks = [(i * CH) for i in range(SLOTS // CH)]
    if SLOTS % CH:
        chunks.append(SLOTS - (SLOTS % CH))

    with tile.TileContext(nc) as tc:
        with (
            tc.tile_pool(name="const", bufs=1) as constp,
            tc.tile_pool(name="ld", bufs=3) as ldp,
            tc.tile_pool(name="ps", bufs=1, space="PSUM") as psp,
            tc.tile_pool(name="wk", bufs=3) as wkp,
        ):
            ones_row = constp.tile([1, 128], F32)
            nc.vector.memset(ones_row[:], 1.0)
            acr_sb = constp.tile([1, 2 * HD], F32)
            nc.sync.dma_start(out=acr_sb[:], in_=acr[:])
            ac_ps = psp.tile([128, 2 * HD], F32)
            nc.tensor.matmul(out=ac_ps[:], lhsT=ones_row[:], rhs=acr_sb[:],
                             start=True, stop=True)
            abc = constp.tile([128, 2 * HD], F32)
            nc.scalar.activation(abc[:], ac_ps[:],
                                 mybir.ActivationFunctionType.Copy)
            a_b = (abc[:, 0:HD].rearrange("p (o c) -> p o c", o=1)
                   .to_broadcast([128, NT, HD]))
            c_b = (abc[:, HD:2 * HD].rearrange("p (o c) -> p o c", o=1)
                   .to_broadcast([128, NT, HD]))

            qs = [nc.sync, nc.gpsimd, nc.scalar]
            for i, o in enumerate(chunks):
                w = min(CH, SLOTS - o)
                nt = w // 128
                hl = ldp.tile([128, CH], F32, tag="hl")
                qs[i % 3].dma_start(
                    out=hl[:, :w].rearrange("p (f c) -> p f c", c=HD),
                    in_=h_in[o:o + w, :].rearrange("(f p) c -> p f c", f=nt))
                h2 = wkp.tile([128, CH], F16, tag="h2")
                h2v = h2[:, :w].rearrange("p (f c) -> p f c", c=HD)
                nc.vector.tensor_tensor(
                    out=h2v, in0=hl[:, :w].rearrange("p (f c) -> p f c",
                                                     c=HD),
                    in1=a_b[:, 0:nt], op=mybir.AluOpType.mult)
                nc.vector.tensor_tensor(
                    out=h2v, in0=h2v, in1=c_b[:, 0:nt],
                    op=mybir.AluOpType.add)
                m = wkp.tile([128, CH], F16, tag="m")
                nc.vector.tensor_scalar(out=m[:, :w], in0=h2[:, :w],
                                        scalar1=0.0, scalar2=None,
                                        op0=mybir.AluOpType.min)
                nc.scalar.activation(m[:, :w], m[:, :w],
                                     mybir.ActivationFunctionType.Exp)
                nc.vector.tensor_scalar(out=m[:, :w], in0=m[:, :w],
                                        scalar1=-1.0, scalar2=None,
                                        op0=mybir.AluOpType.add)
                # elu = max(h2, exp(min(h2,0))-1)
                nc.vector.tensor_tensor(out=h2[:, :w], in0=h2[:, :w],
                                        in1=m[:, :w],
                                        op=mybir.AluOpType.max)
                xt = ldp.tile([128, CH], F32, tag="xt")
                qs[(i + 1) % 3].dma_start(
                    out=xt[:, :w].rearrange("p (f c) -> p f c", c=HD),
                    in_=x_rows[o:o + w, :].rearrange("(f p) c -> p f c",
                                                     f=nt))
                ho = wkp.tile([128, CH], F32, tag="ho")
                nc.vector.tensor_tensor(out=ho[:, :w], in0=h2[:, :w],
                                        in1=xt[:, :w],
                                        op=mybir.AluOpType.add)
                qs[(i + 2) % 3].dma_start(
                    out=out_r[o:o + w, :].rearrange("(f p) c -> p f c",
                                                    f=nt),
                    in_=ho[:, :w].rearrange("p (f c) -> p f c", c=HD))

    nc.compile()
    return nc


def _host_prep(src, dst):
    """Degree-sorted slot assignment + per-edge column placement.

    Returns (per_core, caps, coloff):
      per_core[c] = (colsrc [totcol, 128] int32 index into xext columns,
                     node_of_slot [SLOTS] int64, -1 for pads)
      caps [NBLK], coloff [NBLK+1]
    """
    src = src.astype(np.int64)
    dst = dst.astype(np.int64)
    tmp = []
    caps_all = np.zeros((NCORES, NBLK), np.int64)
    for c in range(NCORES):
        lo = c * NSHARD
        m = (dst >= lo) & (dst < lo + NSHARD)
        es = src[m]
        ed = dst[m] - lo
        deg = np.bincount(ed, minlength=NSHARD)
        order = np.argsort(-deg, kind="stable")
        rank = np.empty(NSHARD, np.int64)
        rank[order] = np.arange(NSHARD)
        dpad = np.zeros(SLOTS, np.int64)
        dpad[:NSHARD] = deg[order]
        caps_all[c] = dpad.reshape(NBLK, 128).max(axis=1)
        r_e = rank[ed]
        eorder = np.argsort(r_e, kind="stable")
        r_sorted = r_e[eorder]
        s_sorted = es[eorder]
        starts = np.searchsorted(r_sorted, np.arange(NSHARD))
        k_e = np.arange(len(r_sorted)) - starts[r_sorted]
        tmp.append((order, dpad, r_sorted, s_sorted, k_e))
    caps = caps_all.max(axis=0)
    caps = np.maximum(((caps + 2) // 3) * 3, 3)
    coloff = np.zeros(NBLK + 1, np.int64)
    np.cumsum(caps, out=coloff[1:])
    totcol = int(coloff[-1])

    per_core = []
    for c in range(NCORES):
        order, dpad, r_sorted, s_sorted, k_e = tmp[c]
        node_of_slot = np.full(SLOTS, -1, np.int64)
        node_of_slot[:NSHARD] = order + c * NSHARD
        # defaults: v-pad (col N) where slot has edges, zero (col N+1) else
        defaults = np.where(dpad.reshape(NBLK, 128) > 0, N, N + 1)
        colsrc = np.repeat(defaults, caps, axis=0).astype(np.int32)
        b_e = r_sorted // 128
        s_e = r_sorted % 128
        colpos = coloff[b_e] + k_e
        colsrc[colpos, s_e] = s_sorted.astype(np.int32)
        per_core.append((colsrc, node_of_slot))
    return per_core, caps, coloff


def kernel(x, src, dst, W, attn_l, attn_r, bias, gamma, beta):
    global LAST_EXEC_NS
    x = np.asarray(x, np.float32)
    src = np.asarray(src, np.int32)
    dst = np.asarray(dst, np.int32)
    W = np.asarray(W, np.float32)
    attn_l = np.asarray(attn_l, np.float32)
    attn_r = np.asarray(attn_r, np.float32)
    gamma = np.asarray(gamma, np.float32)
    beta = np.asarray(beta, np.float32)

    per_core, caps, coloff = _host_prep(src, dst)

    key = ("l1", caps.tobytes())
    if key not in _cache:
        _cache[key] = _build_launch1(caps)
    if "l2" not in _cache:
        _cache["l2"] = _build_launch2()
    nc1, nc2 = _cache[key], _cache["l2"]

    almat = np.zeros((HD, H), np.float32)
    armat = np.zeros((HD, H), np.float32)
    for h in range(H):
        almat[h * D:(h + 1) * D, h] = attn_l[h]
        armat[h * D:(h + 1) * D, h] = attn_r[h]
    Wl = W @ almat                                       # [128, 8]
    Wf = np.concatenate([W, Wl], axis=1).astype(np.float16)   # [128, 136]
    Wr = (W @ armat).astype(np.float16)                  # [128, 8]
    # pad vector: Wl^T v = ELPAD * ones  (least squares, exact: 8 <= 128)
    G = Wl.T @ Wl
    v = Wl @ np.linalg.solve(G, np.full(H, ELPAD, np.float64)).astype(
        np.float32)
    xext = np.concatenate(
        [x.T.astype(np.float16),
         v[:, None].astype(np.float16),
         np.zeros((HD, 1), np.float16)], axis=1)         # [128, N+2]

    in_maps = []
    xrows_list = []
    for c in range(NCORES):
        colsrc, node_of_slot = per_core[c]
        xe = xext[:, colsrc.reshape(-1)]                 # [128, totcol*128]
        xr = np.zeros((SLOTS, HD), np.float32)
        real = node_of_slot >= 0
        xr[real] = x[node_of_slot[real]]
        xrows_list.append(xr)
        in_maps.append({
            "xe": xe, "xTp16": xr.T.astype(np.float16),
            "Wf": Wf, "Wr": Wr,
        })

    res1 = run_bass_kernel_spmd(nc1, in_maps, list(range(NCORES)),
                                **_trace_kwargs())
    LAST_EXEC_NS[0] = res1.exec_time_ns or 0

    # host: combine BN stats from h (pad slots are exactly zero)
    S1 = np.zeros(128, np.float64)
    S2 = np.zeros(128, np.float64)
    for c in range(NCORES):
        hv = res1.results[c]["h_out"].astype(np.float64)
        S1 += hv.sum(axis=0)
        S2 += (hv * hv).sum(axis=0)
    mu = (S1 / N).astype(np.float32)
    var = (S2 / N - (S1 / N) ** 2).astype(np.float32)
    a = gamma / np.sqrt(var + EPS)
    cc = beta - a * mu
    ac = np.stack([a, cc], axis=1).astype(np.float32)

    acr = np.concatenate([a, cc]).reshape(1, 2 * HD).astype(np.float32)
    in_maps2 = []
    for c in range(NCORES):
        in_maps2.append({
            "h_in": res1.results[c]["h_out"],
            "x_rows": xrows_list[c],
            "acr": acr,
        })
    res2 = run_bass_kernel_spmd(nc2, in_maps2, list(range(NCORES)),
                                **_trace_kwargs())
    LAST_EXEC_NS[1] = res2.exec_time_ns or 0

    out = np.zeros((N, IN_DIM), np.float32)
    for c in range(NCORES):
        node_of_slot = per_core[c][1]
        real = node_of_slot >= 0
        orow = res2.results[c]["out_r"]
        out[node_of_slot[real]] = orow[real]
    return out


def _trace_kwargs():
    import os
    if os.environ.get("GAT_TRACE", "0") == "1":
        return {"trace": True}
    return {}
